# revision 7
# baseline (speedup 1.0000x reference)
"""Causal attention with key padding for Trainium2, sharded over 8 NeuronCores.

Contract: kernel(**inputs) takes the FULL inputs (q, k, v, att_mask, pad_mask)
as numpy arrays and returns the FULL [B, H, L, D] output.

Strategy:
  - Shard the 64 (batch, head) units across 8 cores: core c gets units
    [8c, 8c+8), so each core sees a single batch's pad mask.
  - Host pre-transposes Q and K to [unit, D, L] (bf16) so the device never
    transposes activations for the d-contracted matmuls.
  - Per head and per 512-wide query block we compute S^T[k, q] (keys on
    partitions) with bf16 matmuls (fp32 PSUM accumulate), exp() on ScalarE
    directly out of PSUM (scale=1/sqrt(D) fused into the activation, bf16
    out), apply the causal boundary with one precomputed [128,128] triangle
    mask multiply per crossing chunk, apply key padding with a data-driven
    0/1 per-partition multiply, and accumulate P~ @ V with a ones-row folded
    into V so the softmax denominators come out of the same matmuls.  A PE
    transpose + per-query reciprocal multiply then yields normalized fp32
    output in natural [q, d] layout which DMAs straight to DRAM.
"""

import numpy as np

N_CORES = 8
KC = 128          # key-chunk (partition) size
QB = 512          # query-block width
MAX_BATCH = 3     # S^T chunks per exp batch (<= 3 PSUM banks)


# --------------------------------------------------------------------------
# numpy fallback (exact reference math) -- only used if the input masks do
# not match the causal + suffix-pad structure this kernel specializes to.
# --------------------------------------------------------------------------
def _reference_np(q, k, v, att_mask, pad_mask):
    B, H, L, D = q.shape
    scale = np.float32(1.0) / np.sqrt(np.float32(D))
    out = np.empty_like(q)
    for b in range(B):
        for h in range(H):
            att = (q[b, h] @ k[b, h].T) * scale
            att = att + att_mask[0, 0]
            att = np.where(pad_mask[b][None, :], -np.inf, att)
            att = att - att.max(axis=-1, keepdims=True)
            p = np.exp(att)
            p = p / p.sum(axis=-1, keepdims=True)
            out[b, h] = p @ v[b, h]
    return out


# --------------------------------------------------------------------------
# Bass program builder
# --------------------------------------------------------------------------
def _build_program(NH, L, D, chunk_status):
    """Build the per-core SPMD Bass program.

    NH: heads per core.  L: sequence length.  D: head dim (<= 128, contract).
    chunk_status: list over key chunks of 'valid' | 'partial' | 'skip'.
    """
    import concourse.bacc as bacc
    import concourse.mybir as mybir
    import concourse.tile as tile

    f32 = mybir.dt.float32
    bf16 = mybir.dt.bfloat16
    NCH = L // KC
    NQB = L // QB
    CPB = QB // KC  # chunks spanning one query block (crossing chunks)
    scale = float(1.0 / np.sqrt(np.float32(D)))

    nc = bacc.Bacc("TRN2", target_bir_lowering=False, debug=False)

    qt_d = nc.dram_tensor("qt", [NH, D, L], bf16, kind="ExternalInput")
    kt_d = nc.dram_tensor("kt", [NH, D, L], bf16, kind="ExternalInput")
    v_d = nc.dram_tensor("v", [NH, L, D], bf16, kind="ExternalInput")
    pad_d = nc.dram_tensor("pad01", [KC, NCH], f32, kind="ExternalInput")
    tri_d = nc.dram_tensor("trimask", [KC, KC], bf16, kind="ExternalInput")
    id_d = nc.dram_tensor("ident", [KC, KC], f32, kind="ExternalInput")
    out_d = nc.dram_tensor("out", [NH, L, D], f32, kind="ExternalOutput")

    with tile.TileContext(nc) as tc:
        with (
            tc.tile_pool(name="consts", bufs=1) as consts,
            tc.tile_pool(name="ktp", bufs=2) as ktp,
            tc.tile_pool(name="qtp", bufs=2) as qtp,
            tc.tile_pool(name="vop", bufs=2) as vop,
            tc.tile_pool(name="ptp", bufs=3) as ptp,
            tc.tile_pool(name="accsb", bufs=2) as accsb,
            tc.tile_pool(name="osb", bufs=2) as osb,
            tc.tile_pool(name="recp", bufs=4) as recp,
            tc.tile_pool(name="stg", bufs=2, space="PSUM") as stgp,
            tc.tile_pool(name="acc", bufs=1, space="PSUM") as accp,
            tc.tile_pool(name="pst", bufs=1, space="PSUM") as pstp,
        ):
            pad01 = consts.tile([KC, NCH], f32)
            tri = consts.tile([KC, KC], bf16)
            ident = consts.tile([KC, KC], f32)
            nc.sync.dma_start(out=pad01[:], in_=pad_d[:])
            nc.sync.dma_start(out=tri[:], in_=tri_d[:])
            nc.sync.dma_start(out=ident[:], in_=id_d[:])

            for h in range(NH):
                kt_t = ktp.tile([D, L], bf16)
                qt_t = qtp.tile([D, L], bf16)
                vo_t = vop.tile([KC, NCH, D + 1], bf16)
                nc.sync.dma_start(out=kt_t[:], in_=kt_d[h])
                nc.sync.dma_start(out=qt_t[:], in_=qt_d[h])
                nc.gpsimd.dma_start(
                    out=vo_t[:, :, 0:D],
                    in_=v_d[h].rearrange("(c p) d -> p c d", p=KC),
                )
                nc.vector.memset(vo_t[:, :, D], 1.0)
                o_sb = osb.tile([KC, L // KC, D], f32)

                # Software-pipelined emission: PV for batch i is emitted
                # after QK^T+exp of batch i+1 so the PE never sits behind a
                # PV that is waiting on ScalarE; each query block's epilogue
                # is emitted after the next block's first QK^T batch.
                pv_queue = []      # (acc, pt, batch, qb, first, last)
                pending_epi = []

                def emit_pv(job):
                    acc, pt, batch, qb, first_kc, last_kc = job
                    for i, kc in enumerate(batch):
                        base = i * QB
                        j = kc - CPB * qb
                        qs = j * KC if j >= 0 else 0
                        nc.tensor.matmul(
                            out=acc[:, qs:QB],
                            lhsT=vo_t[:, kc, :],
                            rhs=pt[:, base + qs : base + QB],
                            start=(kc == first_kc),
                            stop=(kc == last_kc),
                        )

                def emit_epi(qb, acc):
                    acs = accsb.tile([D + 1, QB], f32)
                    nc.vector.tensor_copy(out=acs[:], in_=acc[:])
                    pst = pstp.tile([KC, QB // KC, D + 1], f32)
                    for half in range(QB // KC):
                        nc.tensor.transpose(
                            out=pst[:, half, :],
                            in_=acs[:, half * KC : (half + 1) * KC],
                            identity=ident[0 : D + 1, 0 : D + 1],
                        )
                    rec = recp.tile([KC, QB // KC], f32)
                    nc.vector.reciprocal(out=rec[:], in_=pst[:, :, D])
                    for half in range(QB // KC):
                        nc.vector.tensor_scalar_mul(
                            o_sb[:, (QB // KC) * qb + half, :],
                            pst[:, half, 0:D],
                            rec[:, half : half + 1],
                        )

                for qb in range(NQB):
                    chunks = [
                        kc
                        for kc in range(min(CPB * (qb + 1), NCH))
                        if chunk_status[kc] != "skip"
                    ]
                    acc = accp.tile([D + 1, QB], f32)
                    first_kc, last_kc = chunks[0], chunks[-1]

                    batches = [
                        chunks[i : i + MAX_BATCH]
                        for i in range(0, len(chunks), MAX_BATCH)
                    ]

                    for bi, batch in enumerate(batches):
                        nb = len(batch)
                        stg = stgp.tile([KC, MAX_BATCH * QB], f32)
                        for i, kc in enumerate(batch):
                            nc.tensor.matmul(
                                out=stg[:, i * QB : (i + 1) * QB],
                                lhsT=kt_t[:, kc * KC : (kc + 1) * KC],
                                rhs=qt_t[:, qb * QB : (qb + 1) * QB],
                                start=True,
                                stop=True,
                            )
                        pt = ptp.tile([KC, MAX_BATCH * QB], bf16)
                        nc.scalar.activation(
                            out=pt[:, 0 : nb * QB],
                            in_=stg[:, 0 : nb * QB],
                            func=mybir.ActivationFunctionType.Exp,
                            scale=scale,
                        )
                        # causal boundary + key padding fixups
                        for i, kc in enumerate(batch):
                            base = i * QB
                            if kc >= CPB * qb:  # crossing chunk
                                qs = (kc - CPB * qb) * KC
                                nc.vector.tensor_mul(
                                    out=pt[:, base + qs : base + qs + KC],
                                    in0=pt[:, base + qs : base + qs + KC],
                                    in1=tri[:],
                                )
                            if chunk_status[kc] == "partial":
                                nc.vector.tensor_scalar_mul(
                                    pt[:, base : base + QB],
                                    pt[:, base : base + QB],
                                    pad01[:, kc : kc + 1],
                                )
                        if bi == 0 and pending_epi:
                            emit_epi(*pending_epi.pop())
                        while pv_queue:
                            emit_pv(pv_queue.pop(0))
                        pv_queue.append((acc, pt, batch, qb, first_kc, last_kc))
                    # drain this block's last PV so the accumulator is
                    # complete; its epilogue is deferred into the next block
                    while pv_queue:
                        emit_pv(pv_queue.pop(0))
                    pending_epi.append((qb, acc))
                if pending_epi:
                    emit_epi(*pending_epi.pop())
                nc.gpsimd.dma_start(
                    out=out_d[h].rearrange("(j p) d -> p j d", p=KC),
                    in_=o_sb[:],
                )
    nc.finalize()
    return nc


# --------------------------------------------------------------------------
# host-side wrapper
# --------------------------------------------------------------------------
_PROG_CACHE = {}


def _get_program(NH, L, D, chunk_status):
    key = (NH, L, D, tuple(chunk_status))
    if key not in _PROG_CACHE:
        _PROG_CACHE[key] = _build_program(NH, L, D, chunk_status)
    return _PROG_CACHE[key]


def _causal_ok(att_mask, L):
    if att_mask.shape != (1, 1, L, L):
        return False
    m = att_mask[0, 0]
    iu = np.triu_indices(L, 1)
    if not np.all(m[iu] == np.float32(-1e9)):
        return False
    il = np.tril_indices(L)
    return bool(np.all(m[il] == 0.0))


def kernel(q, k, v, att_mask, pad_mask):
    import ml_dtypes

    from concourse.bass_utils import run_bass_kernel_spmd

    B, H, L, D = q.shape
    U = B * H
    if (
        U % N_CORES != 0
        or L % QB != 0
        or D > KC
        or not _causal_ok(att_mask, L)
    ):
        return _reference_np(q, k, v, att_mask, pad_mask)

    NH = U // N_CORES  # units per core
    NCH = L // KC

    # per-unit pad rows; each core must see a single pad row across units.
    pad = np.asarray(pad_mask, dtype=bool)          # [B, L]
    pad_u = np.repeat(pad, H, axis=0)               # [U, L]
    pad_c = pad_u.reshape(N_CORES, NH, L)
    if not all(np.all(pad_c[c] == pad_c[c][0]) for c in range(N_CORES)):
        return _reference_np(q, k, v, att_mask, pad_mask)

    # chunk status must be consistent across cores (single SPMD program)
    chunk_status = []
    for kc in range(NCH):
        sl = pad_u[:, kc * KC : (kc + 1) * KC]
        if np.all(sl):
            chunk_status.append("skip")
        elif not np.any(sl):
            chunk_status.append("valid")
        else:
            chunk_status.append("partial")
    if chunk_status[0] == "skip":
        return _reference_np(q, k, v, att_mask, pad_mask)

    bf = ml_dtypes.bfloat16
    qf = np.ascontiguousarray(
        q.reshape(U, L, D).transpose(0, 2, 1)
    ).astype(bf)
    kf = np.ascontiguousarray(
        k.reshape(U, L, D).transpose(0, 2, 1)
    ).astype(bf)
    vf = np.ascontiguousarray(v.reshape(U, L, D)).astype(bf)

    tri = (np.arange(KC)[None, :] >= np.arange(KC)[:, None]).astype(bf)
    ident = np.eye(KC, dtype=np.float32)

    in_maps = []
    for c in range(N_CORES):
        sl = slice(c * NH, (c + 1) * NH)
        pad01 = (~pad_c[c][0]).astype(np.float32).reshape(NCH, KC).T.copy()
        in_maps.append(
            {
                "qt": qf[sl],
                "kt": kf[sl],
                "v": vf[sl],
                "pad01": np.ascontiguousarray(pad01),
                "trimask": tri,
                "ident": ident,
            }
        )

    nc = _get_program(NH, L, D, chunk_status)
    import os

    kwargs = {}
    if os.environ.get("BASS_KERNEL_PROFILE") == "1":
        kwargs = dict(trace=True, trace_cores=[0], stitch_traces=False)
    res = run_bass_kernel_spmd(nc, in_maps, list(range(N_CORES)), **kwargs)
    global LAST_RESULT
    LAST_RESULT = res
    out = np.concatenate([r["out"] for r in res.results], axis=0)
    return out.reshape(B, H, L, D).astype(q.dtype, copy=False)


LAST_RESULT = None


# revision 9
# speedup vs baseline: 1.3398x; 1.3398x over previous
"""Causal attention with key padding for Trainium2, sharded over 8 NeuronCores.

Contract: kernel(**inputs) takes the FULL inputs (q, k, v, att_mask, pad_mask)
as numpy arrays and returns the FULL [B, H, L, D] output.

Strategy:
  - Shard the 64 (batch, head) units across 8 cores: core c gets units
    [8c, 8c+8), so each core sees a single batch's pad mask.
  - Host pre-transposes Q and K to [unit, D, L] (bf16) so the device never
    transposes activations for the d-contracted matmuls.
  - Per head and per 512-wide query block we compute S^T[k, q] (keys on
    partitions) with bf16 matmuls (fp32 PSUM accumulate), exp() on ScalarE
    directly out of PSUM (scale=1/sqrt(D) fused into the activation, bf16
    out), apply the causal boundary with one precomputed [128,128] triangle
    mask multiply per crossing chunk, apply key padding with a data-driven
    0/1 per-partition multiply, and accumulate P~ @ V with a ones-row folded
    into V so the softmax denominators come out of the same matmuls.  A PE
    transpose + per-query reciprocal multiply then yields normalized fp32
    output in natural [q, d] layout which DMAs straight to DRAM.
"""

import numpy as np

N_CORES = 8
KC = 128          # key-chunk (partition) size
QB = 512          # query-block width
MAX_BATCH = 3     # S^T chunks per exp batch (<= 3 PSUM banks)


# --------------------------------------------------------------------------
# numpy fallback (exact reference math) -- only used if the input masks do
# not match the causal + suffix-pad structure this kernel specializes to.
# --------------------------------------------------------------------------
def _reference_np(q, k, v, att_mask, pad_mask):
    B, H, L, D = q.shape
    scale = np.float32(1.0) / np.sqrt(np.float32(D))
    out = np.empty_like(q)
    for b in range(B):
        for h in range(H):
            att = (q[b, h] @ k[b, h].T) * scale
            att = att + att_mask[0, 0]
            att = np.where(pad_mask[b][None, :], -np.inf, att)
            att = att - att.max(axis=-1, keepdims=True)
            p = np.exp(att)
            p = p / p.sum(axis=-1, keepdims=True)
            out[b, h] = p @ v[b, h]
    return out


# --------------------------------------------------------------------------
# Bass program builder
# --------------------------------------------------------------------------
def _build_program(NH, L, D, chunk_status):
    """Build the per-core SPMD Bass program.

    NH: heads per core.  L: sequence length.  D: head dim (<= 128, contract).
    chunk_status: list over key chunks of 'valid' | 'partial' | 'skip'.
    """
    import concourse.bacc as bacc
    import concourse.mybir as mybir
    import concourse.tile as tile

    f32 = mybir.dt.float32
    bf16 = mybir.dt.bfloat16
    NCH = L // KC
    NQB = L // QB
    CPB = QB // KC  # chunks spanning one query block (crossing chunks)
    scale = float(1.0 / np.sqrt(np.float32(D)))

    nc = bacc.Bacc("TRN2", target_bir_lowering=False, debug=False)

    qt_d = nc.dram_tensor("qt", [NH, D, L], bf16, kind="ExternalInput")
    kt_d = nc.dram_tensor("kt", [NH, D, L], bf16, kind="ExternalInput")
    v_d = nc.dram_tensor("v", [NH, L, D], bf16, kind="ExternalInput")
    pad_d = nc.dram_tensor("pad01", [KC, NCH], f32, kind="ExternalInput")
    tri_d = nc.dram_tensor("trimask", [KC, KC], bf16, kind="ExternalInput")
    id_d = nc.dram_tensor("ident", [KC, KC], f32, kind="ExternalInput")
    out_d = nc.dram_tensor("out", [NH, L, D], f32, kind="ExternalOutput")

    with tile.TileContext(nc) as tc:
        with (
            tc.tile_pool(name="consts", bufs=1) as consts,
            tc.tile_pool(name="ktp", bufs=2) as ktp,
            tc.tile_pool(name="qtp", bufs=2) as qtp,
            tc.tile_pool(name="vop", bufs=2) as vop,
            tc.tile_pool(name="ptp", bufs=3) as ptp,
            tc.tile_pool(name="accsb", bufs=2) as accsb,
            tc.tile_pool(name="osb", bufs=2) as osb,
            tc.tile_pool(name="recp", bufs=4) as recp,
            tc.tile_pool(name="stg", bufs=2, space="PSUM") as stgp,
            tc.tile_pool(name="acc", bufs=1, space="PSUM") as accp,
            tc.tile_pool(name="pst", bufs=1, space="PSUM") as pstp,
        ):
            pad01 = consts.tile([KC, NCH], f32)
            tri = consts.tile([KC, KC], bf16)
            ident = consts.tile([KC, KC], f32)
            nc.sync.dma_start(out=pad01[:], in_=pad_d[:])
            nc.sync.dma_start(out=tri[:], in_=tri_d[:])
            nc.sync.dma_start(out=ident[:], in_=id_d[:])

            for h in range(NH):
                # K^T and Q^T are duplicated into both partition halves so
                # QK^T matmuls can run 2x row-packed (contract dim is D=64,
                # half the PE array) via tile_position row groups.
                kt_t = ktp.tile([2 * D, L], bf16)
                qt_t = qtp.tile([2 * D, L], bf16)
                vo_t = vop.tile([KC, NCH, D + 1], bf16)
                nc.sync.dma_start(out=kt_t[0:D, :], in_=kt_d[h])
                nc.sync.dma_start(out=kt_t[D : 2 * D, :], in_=kt_d[h])
                nc.sync.dma_start(out=qt_t[0:D, :], in_=qt_d[h])
                nc.sync.dma_start(out=qt_t[D : 2 * D, :], in_=qt_d[h])
                nc.gpsimd.dma_start(
                    out=vo_t[:, :, 0:D],
                    in_=v_d[h].rearrange("(c p) d -> p c d", p=KC),
                )
                nc.vector.memset(vo_t[:, :, D], 1.0)
                o_sb = osb.tile([KC, L // KC, D], f32)

                # Software-pipelined emission: PV for batch i is emitted
                # after QK^T+exp of batch i+1 so the PE never sits behind a
                # PV that is waiting on ScalarE; each query block's epilogue
                # is emitted after the next block's first QK^T batch.
                pv_queue = []      # (acc, pt, batch, qb, first, last)
                pending_epi = []

                def emit_pv(job):
                    acc, pt, batch, qb, first_kc, last_kc = job
                    for i, kc in enumerate(batch):
                        base = i * QB
                        j = kc - CPB * qb
                        qs = j * KC if j >= 0 else 0
                        nc.tensor.matmul(
                            out=acc[:, qs:QB],
                            lhsT=vo_t[:, kc, :],
                            rhs=pt[:, base + qs : base + QB],
                            start=(kc == first_kc),
                            stop=(kc == last_kc),
                        )

                def emit_epi(qb, acc):
                    acs = accsb.tile([D + 1, QB], f32)
                    nc.vector.tensor_copy(out=acs[:], in_=acc[:])
                    pst = pstp.tile([KC, QB // KC, D + 1], f32)
                    for half in range(QB // KC):
                        nc.tensor.transpose(
                            out=pst[:, half, :],
                            in_=acs[:, half * KC : (half + 1) * KC],
                            identity=ident[0 : D + 1, 0 : D + 1],
                        )
                    rec = recp.tile([KC, QB // KC], f32)
                    nc.vector.reciprocal(out=rec[:], in_=pst[:, :, D])
                    for half in range(QB // KC):
                        nc.vector.tensor_scalar_mul(
                            o_sb[:, (QB // KC) * qb + half, :],
                            pst[:, half, 0:D],
                            rec[:, half : half + 1],
                        )

                for qb in range(NQB):
                    chunks = [
                        kc
                        for kc in range(min(CPB * (qb + 1), NCH))
                        if chunk_status[kc] != "skip"
                    ]
                    acc = accp.tile([D + 1, QB], f32)
                    first_kc, last_kc = chunks[0], chunks[-1]

                    batches = [
                        chunks[i : i + MAX_BATCH]
                        for i in range(0, len(chunks), MAX_BATCH)
                    ]

                    for bi, batch in enumerate(batches):
                        nb = len(batch)
                        stg = stgp.tile([KC, MAX_BATCH * QB], f32)
                        for i, kc in enumerate(batch):
                            half = i % 2  # row-group for 2x packing
                            nc.tensor.matmul(
                                out=stg[:, i * QB : (i + 1) * QB],
                                lhsT=kt_t[
                                    half * D : (half + 1) * D,
                                    kc * KC : (kc + 1) * KC,
                                ],
                                rhs=qt_t[
                                    half * D : (half + 1) * D,
                                    qb * QB : (qb + 1) * QB,
                                ],
                                start=True,
                                stop=True,
                            )
                        pt = ptp.tile([KC, MAX_BATCH * QB], bf16)
                        nc.scalar.activation(
                            out=pt[:, 0 : nb * QB],
                            in_=stg[:, 0 : nb * QB],
                            func=mybir.ActivationFunctionType.Exp,
                            scale=scale,
                        )
                        # causal boundary + key padding fixups
                        for i, kc in enumerate(batch):
                            base = i * QB
                            if kc >= CPB * qb:  # crossing chunk
                                qs = (kc - CPB * qb) * KC
                                nc.vector.tensor_mul(
                                    out=pt[:, base + qs : base + qs + KC],
                                    in0=pt[:, base + qs : base + qs + KC],
                                    in1=tri[:],
                                )
                            if chunk_status[kc] == "partial":
                                nc.vector.tensor_scalar_mul(
                                    pt[:, base : base + QB],
                                    pt[:, base : base + QB],
                                    pad01[:, kc : kc + 1],
                                )
                        if bi == 0 and pending_epi:
                            emit_epi(*pending_epi.pop())
                        while pv_queue:
                            emit_pv(pv_queue.pop(0))
                        pv_queue.append((acc, pt, batch, qb, first_kc, last_kc))
                    # drain this block's last PV so the accumulator is
                    # complete; its epilogue is deferred into the next block
                    while pv_queue:
                        emit_pv(pv_queue.pop(0))
                    pending_epi.append((qb, acc))
                if pending_epi:
                    emit_epi(*pending_epi.pop())
                nc.gpsimd.dma_start(
                    out=out_d[h].rearrange("(j p) d -> p j d", p=KC),
                    in_=o_sb[:],
                )
    nc.finalize()
    return nc


# --------------------------------------------------------------------------
# host-side wrapper
# --------------------------------------------------------------------------
_PROG_CACHE = {}


def _get_program(NH, L, D, chunk_status):
    key = (NH, L, D, tuple(chunk_status))
    if key not in _PROG_CACHE:
        _PROG_CACHE[key] = _build_program(NH, L, D, chunk_status)
    return _PROG_CACHE[key]


def _causal_ok(att_mask, L):
    if att_mask.shape != (1, 1, L, L):
        return False
    m = att_mask[0, 0]
    iu = np.triu_indices(L, 1)
    if not np.all(m[iu] == np.float32(-1e9)):
        return False
    il = np.tril_indices(L)
    return bool(np.all(m[il] == 0.0))


def kernel(q, k, v, att_mask, pad_mask):
    import ml_dtypes

    from concourse.bass_utils import run_bass_kernel_spmd

    B, H, L, D = q.shape
    U = B * H
    if (
        U % N_CORES != 0
        or L % QB != 0
        or D > KC
        or not _causal_ok(att_mask, L)
    ):
        return _reference_np(q, k, v, att_mask, pad_mask)

    NH = U // N_CORES  # units per core
    NCH = L // KC

    # per-unit pad rows; each core must see a single pad row across units.
    pad = np.asarray(pad_mask, dtype=bool)          # [B, L]
    pad_u = np.repeat(pad, H, axis=0)               # [U, L]
    pad_c = pad_u.reshape(N_CORES, NH, L)
    if not all(np.all(pad_c[c] == pad_c[c][0]) for c in range(N_CORES)):
        return _reference_np(q, k, v, att_mask, pad_mask)

    # chunk status must be consistent across cores (single SPMD program)
    chunk_status = []
    for kc in range(NCH):
        sl = pad_u[:, kc * KC : (kc + 1) * KC]
        if np.all(sl):
            chunk_status.append("skip")
        elif not np.any(sl):
            chunk_status.append("valid")
        else:
            chunk_status.append("partial")
    if chunk_status[0] == "skip":
        return _reference_np(q, k, v, att_mask, pad_mask)

    bf = ml_dtypes.bfloat16
    qf = np.ascontiguousarray(
        q.reshape(U, L, D).transpose(0, 2, 1)
    ).astype(bf)
    kf = np.ascontiguousarray(
        k.reshape(U, L, D).transpose(0, 2, 1)
    ).astype(bf)
    vf = np.ascontiguousarray(v.reshape(U, L, D)).astype(bf)

    tri = (np.arange(KC)[None, :] >= np.arange(KC)[:, None]).astype(bf)
    ident = np.eye(KC, dtype=np.float32)

    in_maps = []
    for c in range(N_CORES):
        sl = slice(c * NH, (c + 1) * NH)
        pad01 = (~pad_c[c][0]).astype(np.float32).reshape(NCH, KC).T.copy()
        in_maps.append(
            {
                "qt": qf[sl],
                "kt": kf[sl],
                "v": vf[sl],
                "pad01": np.ascontiguousarray(pad01),
                "trimask": tri,
                "ident": ident,
            }
        )

    nc = _get_program(NH, L, D, chunk_status)
    import os

    kwargs = {}
    if os.environ.get("BASS_KERNEL_PROFILE") == "1":
        kwargs = dict(trace=True, trace_cores=[0], stitch_traces=False)
    res = run_bass_kernel_spmd(nc, in_maps, list(range(N_CORES)), **kwargs)
    global LAST_RESULT
    LAST_RESULT = res
    out = np.concatenate([r["out"] for r in res.results], axis=0)
    return out.reshape(B, H, L, D).astype(q.dtype, copy=False)


LAST_RESULT = None
